# revision 24
# baseline (speedup 1.0000x reference)
"""Trainium2 Bass kernel for nn_Attention_Block (dense transformer block).

Strategy: pure data-parallel over batch -- 8 samples, 8 NeuronCores, one
sample per core, weights replicated, no collectives. Per core everything
stays channels-on-partitions (c x n layout):

  GN1 (bn_stats + selector-matmul group reduce) -> QKV matmul for q,k (fp8
  DoubleRow, weights x64) -> V^T computed directly as xn^T @ Wv^T (fp8
  DoubleRow, no PE transposes) into fp8 pair-layout tiles with a ones
  column per head for the softmax denominator -> per-head-pair attention:
  QK row-tiled (both heads concurrent on disjoint 64-partition groups),
  exp emitted directly as fp8 e4m3 in DoubleRow pair layout (side A on
  ACT via real Exp with fp8 output; side B on DVE via a Schraudolph
  bit-trick: z = score*log2e + (56 + 8c), round-to-nearest f32->int8
  convert, bits reinterpreted as e4m3), AV as fp8 DoubleRow matmuls
  contracting 256 keys per instruction -> denominator chain (PE column
  transposes -> one parallel reciprocal -> broadcast matmul) -> out-proj
  (fp8 DoubleRow) -> GN2 -> SwiGLU MLP (fp8 DoubleRow both matmuls) ->
  +residual.

Weight matrices are pre-scaled x64 into fp8 e4m3 (descale folded into the
PSUM readout ops); the SwiGLU gate half is scaled x4 with the
compensating 1/4 folded into the MLP2 readout. V's qkv bias is folded
into the out-proj bias host-side (softmax rows sum to 1).
"""

import os

import numpy as np
import ml_dtypes

KSTAGE = int(os.environ.get("KSTAGE", "7"))
EXPBF = int(os.environ.get("EXPBF", "0"))  # 1 = bf16 exp weights fallback

C = 512
NSP = 1024  # 32*32 spatial
CT = 4  # channel tiles of 128
HEADS = 8
D = 64
HID = 2048
EPS = 1e-5
WS = 64.0  # fp8 weight scale
GS = 4.0  # extra scale on the SwiGLU gate half
LOG2E = 1.4426950408889634
# fp8(e4m3) schraudolph: bits = round(score*log2e + 8*(7 + c)), c calibrated
CEXP8 = 0.095
EXPA8 = LOG2E
EXPB8 = 8.0 * (7.0 + CEXP8)
# bf16 fallback schraudolph (baseline constants)
CEXP = -0.0505
EXPA16 = 0.125 * 128.0 * LOG2E
EXPB16 = 128.0 * (127.0 + CEXP) + 8388608.0

_cache = {}


def _patch_tile_drain(tile, mybir):
    """walrus in this environment accepts very few sync waits per
    instruction; the TileContext tail drain carries one wait per proc of
    the global clock. Split them across preceding SP drains."""
    if getattr(tile.TileContext, "_drain_patched", False):
        return

    def _patched(self, tick_clock, wait_clock):
        nc = self.nc
        spills = [nc.sync.drain() for _ in range(40)]
        drain_inst = nc.sync.drain()
        wait_clock.add_sem_waits(
            drain_inst.ins, tile.ScopedClock({None: tick_clock.global_clock})
        )
        si = drain_inst.ins.sync_info
        waits = list(si.on_wait) if si is not None and si.on_wait else []
        upds = list(si.on_update) if si is not None and si.on_update else []
        if len(waits) > 1:
            *pre, last = waits
            assert len(pre) <= len(spills), "too many drain wait chunks"
            for sp_inst, w in zip(spills, pre):
                sp_inst.ins.sync_info = mybir.SyncInfo(on_wait=[w], on_update=[])
            drain_inst.ins.sync_info = mybir.SyncInfo(on_wait=[last], on_update=upds)
        nc.all_engine_barrier()
        assert self.sems is not None
        popped = nc._tile_sem_poison_stack.pop()
        assert popped is self._sem_poison
        nc.clear_and_free_semaphores(list(self.sems.allocated().values()))
        nc.all_engine_barrier()

    tile.TileContext._drain_and_barrier = _patched
    tile.TileContext._drain_patched = True


def _split_multi_waits(nc, mybir, maxw=1):
    """Hoist extra sync waits onto same-engine EventSemaphore carriers so
    no instruction carries more than `maxw` waits."""
    f = nc.m.functions[0]
    for bb in f.blocks:
        insts = list(bb.instructions)
        need = [
            i
            for i in insts
            if getattr(i, "sync_info", None)
            and i.sync_info.on_wait
            and len(i.sync_info.on_wait) > maxw
        ]
        if not need:
            continue
        carriers = {}
        for inst in need:
            w = list(inst.sync_info.on_wait)
            upds = list(inst.sync_info.on_update) if inst.sync_info.on_update else []
            keep = w[-maxw:]
            extra = w[:-maxw]
            cs = []
            for i in range(0, len(extra), maxw):
                c = mybir.InstEventSemaphore(
                    name=f"I-waitc-{nc.next_id()}", ins=[], outs=[]
                )
                c.engine = inst.engine
                c.sync_info = mybir.SyncInfo(on_wait=extra[i : i + maxw], on_update=[])
                nc.register_instruction(c)
                cs.append(c)
            inst.sync_info = mybir.SyncInfo(on_wait=keep, on_update=upds)
            carriers[inst.name] = cs
        carrier_names = {c.name for cs in carriers.values() for c in cs}
        rebuilt = []
        for inst in list(bb.instructions):
            if inst.name in carrier_names:
                continue
            if inst.name in carriers:
                rebuilt.extend(carriers[inst.name])
            rebuilt.append(inst)
        bb.instructions = rebuilt


def _build_nc():
    import concourse.bass as bass
    import concourse.tile as tile
    from concourse import mybir

    _patch_tile_drain(tile, mybir)

    F32 = mybir.dt.float32
    BF16 = mybir.dt.bfloat16
    FP8 = mybir.dt.float8e4
    I8 = mybir.dt.int8
    ADD = mybir.AluOpType.add
    SUB = mybir.AluOpType.subtract
    MULT = mybir.AluOpType.mult
    AF = mybir.ActivationFunctionType
    DR = mybir.MatmulPerfMode.DoubleRow

    nc = bass.Bass()

    x_d = nc.declare_dram_parameter("x", [C, NSP], F32, isOutput=False)
    # fp8 pair-layout weights: [128, 2*cols]; [q, 2s+m] = W.T[256p+128s+q, m]
    wqk_d = [
        nc.declare_dram_parameter(f"wqkT{p}", [128, 2 * 2 * C], FP8, isOutput=False)
        for p in range(2)
    ]
    wv_d = [
        nc.declare_dram_parameter(f"wvT{p}", [128, 2 * C], FP8, isOutput=False)
        for p in range(2)
    ]
    qkb_d = nc.declare_dram_parameter("qkb", [128, 8], F32, isOutput=False)
    wo_d = [
        nc.declare_dram_parameter(f"woT{p}", [128, 2 * C], FP8, isOutput=False)
        for p in range(2)
    ]
    outb_d = nc.declare_dram_parameter("outb", [128, 4], F32, isOutput=False)
    g1_d = nc.declare_dram_parameter("g1", [128, 4], F32, isOutput=False)
    b1_d = nc.declare_dram_parameter("b1", [128, 4], F32, isOutput=False)
    g2_d = nc.declare_dram_parameter("g2", [128, 4], F32, isOutput=False)
    b2_d = nc.declare_dram_parameter("b2", [128, 4], F32, isOutput=False)
    w1_d = [
        nc.declare_dram_parameter(f"w1T{p}", [128, 2 * 2 * HID], FP8, isOutput=False)
        for p in range(2)
    ]
    w2_d = [
        nc.declare_dram_parameter(f"w2T{p}", [128, 2 * C], FP8, isOutput=False)
        for p in range(8)
    ]
    sel_d = nc.declare_dram_parameter("sel", [C, 32], F32, isOutput=False)
    selT_d = nc.declare_dram_parameter("selT", [32, C], F32, isOutput=False)
    id_d = nc.declare_dram_parameter("ident", [128, 128], BF16, isOutput=False)
    selbc_d = nc.declare_dram_parameter("selbc", [16, 1024], BF16, isOutput=False)
    out_d = nc.declare_dram_parameter("out", [C, NSP], F32, isOutput=True)

    EW = BF16 if EXPBF else FP8  # exp-weight / V dtype

    with tile.TileContext(nc) as tc:
        with (
            tc.tile_pool(name="pers", bufs=1) as pers,
            tc.tile_pool(name="gnp", bufs=2) as gnp,
            tc.tile_pool(name="exq", bufs=2) as exq,
            tc.tile_pool(name="unp", bufs=2) as unp,
            tc.tile_pool(name="invp", bufs=2) as invp,
            tc.tile_pool(name="swp", bufs=2) as swp,
            tc.tile_pool(name="ps", bufs=2, space="PSUM") as ps_pool,
        ):
            # PSUM budget (8 banks): tag "ps" = 2-bank slots x3 (big f32
            # [128,1024] tiles: QKV/MLP/out-proj psums + attention score
            # tiles -- 3 slots so QK of round r+1 overlaps exp of round
            # r), tag "sm" = 1-bank x2 (av/pv/pdt/ptv/pinvb/gn).
            def pstile(shape, dtype, tag="ps", bufs=None):
                if bufs is None:
                    bufs = 3 if tag == "ps" else 2
                return ps_pool.tile(shape, dtype, tag=tag, name=tag, bufs=bufs)

            # ---- input loads (x lands in the attn2 slots; reloaded later) ----
            x_sb = []
            x_engs = [nc.sync, nc.scalar, nc.sync, nc.scalar]
            for t in range(CT):
                xt = pers.tile([128, NSP], F32, tag=f"attn2{t}", name=f"attn2{t}")
                x_engs[t].dma_start(xt[:], x_d[t * 128 : (t + 1) * 128, :])
                x_sb.append(xt)
            sel_sb = []
            for t in range(CT):
                st = pers.tile([128, 32], F32, tag=f"sel{t}", name=f"sel{t}")
                nc.sync.dma_start(st[:], sel_d[t * 128 : (t + 1) * 128, :])
                sel_sb.append(st)
            selT_sb = pers.tile([32, C], F32, tag="selT", name="selT")
            nc.sync.dma_start(selT_sb[:], selT_d[:, :])
            g1_sb = pers.tile([128, 4], F32, tag="g1", name="g1")
            nc.sync.dma_start(g1_sb[:], g1_d[:, :])
            b1_sb = pers.tile([128, 4], F32, tag="b1", name="b1")
            nc.sync.dma_start(b1_sb[:], b1_d[:, :])
            wqk_sb = []
            for p in range(2):
                wt = pers.tile([128, 2, 2 * C], FP8, tag=f"wqk{p}", name=f"wqk{p}")
                nc.sync.dma_start(wt[:].rearrange("p a b -> p (a b)"), wqk_d[p][:, :])
                wqk_sb.append(wt)
            wv_sb = []
            for p in range(2):
                wt = pers.tile([128, 2, C], FP8, tag=f"wv{p}", name=f"wv{p}")
                nc.scalar.dma_start(wt[:].rearrange("p a b -> p (a b)"), wv_d[p][:, :])
                wv_sb.append(wt)
            qkb_sb = pers.tile([128, 8], F32, tag="qkb", name="qkb")
            nc.sync.dma_start(qkb_sb[:], qkb_d[:, :])
            selbc_sb = pers.tile([16, 1024], BF16, tag="selbc", name="selbc")
            nc.sync.dma_start(selbc_sb[:], selbc_d[:, :])
            id_sb = pers.tile([128, 128], BF16, tag="ident", name="ident")
            nc.sync.dma_start(id_sb[:], id_d[:, :])
            wo_sb = [
                pers.tile([128, 2, C], FP8, tag=f"wo{p}", name=f"wo{p}")
                for p in range(2)
            ]
            outb_sb = pers.tile([128, 4], F32, tag="outb", name="outb")
            g2_sb = pers.tile([128, 4], F32, tag="g2", name="g2")
            b2_sb = pers.tile([128, 4], F32, tag="b2", name="b2")
            w1_sb = [
                pers.tile([128, 2, 2 * HID], FP8, tag=f"w1{p}", name=f"w1{p}")
                for p in range(2)
            ]
            w2_sb = [
                pers.tile([128, 2, C], FP8, tag=f"w2{p}", name=f"w2{p}")
                for p in range(8)
            ]

            def load_late_weights():
                # issued after attention emission: keeps early HBM
                # bandwidth for x and the attention-phase inputs
                for p in range(2):
                    nc.sync.dma_start(
                        wo_sb[p][:].rearrange("p a b -> p (a b)"), wo_d[p][:, :]
                    )
                nc.sync.dma_start(outb_sb[:], outb_d[:, :])
                nc.sync.dma_start(g2_sb[:], g2_d[:, :])
                nc.sync.dma_start(b2_sb[:], b2_d[:, :])
                for p in range(2):
                    nc.sync.dma_start(
                        w1_sb[p][:].rearrange("p a b -> p (a b)"), w1_d[p][:, :]
                    )
                for p in range(8):
                    nc.sync.dma_start(
                        w2_sb[p][:].rearrange("p a b -> p (a b)"), w2_d[p][:, :]
                    )

            eps32 = pers.tile([32, 1], F32, tag="eps", name="eps")
            nc.vector.memset(eps32[:], EPS)
            warm = pers.tile([1, 1], F32, tag="warm", name="warm")
            nc.vector.memset(warm[:], 1.0)
            nc.scalar.activation(warm[:], warm[:], AF.Exp)
            magic = pers.tile([32, 1], mybir.dt.int32, tag="magic", name="magic")
            nc.vector.memset(magic[:], 1597463007)
            ones65 = pers.tile([65, 1], BF16, tag="ones65", name="ones65")
            nc.vector.memset(ones65[:], 1.0)

            # ---- group norm helper (32 groups of 16 channels x 1024) ----
            def group_norm(src_tiles, gam_sb, bet_sb, dst_aps):
                rhs3 = []
                for t in range(CT):
                    stats = gnp.tile([128, 2, 6], F32, tag="gn_stats", name="gn_stats")
                    for j2 in range(2):
                        nc.vector.bn_stats(
                            stats[:, j2, :], src_tiles[t][:, j2 * 512 : (j2 + 1) * 512]
                        )
                    mv = gnp.tile([128, 2], F32, tag="gn_mv", name="gn_mv")
                    nc.vector.bn_aggr(mv[:], stats[:])
                    r3 = gnp.tile([128, 3], F32, tag=f"gn_r3_{t}", name=f"gn_r3_{t}")
                    nc.vector.tensor_copy(r3[:, 0:2], mv[:])
                    nc.vector.tensor_mul(r3[:, 2:3], mv[:, 0:1], mv[:, 0:1])
                    rhs3.append(r3)
                pg = pstile([32, 3], F32, tag="sm")
                for t in range(CT):
                    nc.tensor.matmul(
                        pg[:], sel_sb[t][:], rhs3[t][:], start=(t == 0), stop=(t == 3)
                    )
                gs = gnp.tile([32, 2], F32, tag="gn_gs", name="gn_gs")
                tmp = gnp.tile([32, 2], F32, tag="gn_tmp", name="gn_tmp")
                pgs = gnp.tile([32, 3], F32, tag="gn_pgs", name="gn_pgs")
                nc.vector.tensor_copy(pgs[:], pg[:])
                # mean_g, E[x^2]_g, var_g, rstd_g
                nc.vector.tensor_scalar_mul(gs[:, 0:1], pgs[:, 0:1], 1.0 / 16)
                nc.vector.tensor_tensor(tmp[:, 0:1], pgs[:, 1:2], pgs[:, 2:3], op=ADD)
                nc.vector.tensor_scalar_mul(tmp[:, 0:1], tmp[:, 0:1], 1.0 / 16)
                nc.vector.tensor_mul(tmp[:, 1:2], gs[:, 0:1], gs[:, 0:1])
                nc.vector.tensor_tensor(tmp[:, 0:1], tmp[:, 0:1], tmp[:, 1:2], op=SUB)
                # rstd via quake rsqrt + 1 Newton step (max rel err ~0.2%)
                ve = gnp.tile([32, 1], F32, tag="gn_ve", name="gn_ve")
                nc.vector.tensor_scalar_add(ve[:], tmp[:, 0:1], EPS)
                ve2 = gnp.tile([32, 1], F32, tag="gn_ve2", name="gn_ve2")
                nc.vector.tensor_scalar_mul(ve2[:], ve[:], 0.5)
                yb = gnp.tile([32, 1], mybir.dt.int32, tag="gn_yb", name="gn_yb")
                nc.vector.tensor_scalar(
                    yb[:], ve[:].bitcast(mybir.dt.int32),
                    scalar1=1, scalar2=None,
                    op0=mybir.AluOpType.logical_shift_right,
                )
                y0 = gnp.tile([32, 1], mybir.dt.int32, tag="gn_y0", name="gn_y0")
                nc.vector.tensor_tensor(y0[:], magic[:], yb[:], op=SUB)
                ycur = y0[:].bitcast(F32)
                aa = gnp.tile([32, 1], F32, tag="gn_a0", name="gn_a0")
                nc.vector.tensor_mul(aa[:], ycur, ycur)
                nc.vector.tensor_mul(aa[:], aa[:], ve2[:])
                nc.vector.tensor_scalar(
                    aa[:], aa[:], scalar1=-1.0, scalar2=1.5, op0=MULT, op1=ADD
                )
                nc.vector.tensor_mul(gs[:, 1:2], ycur, aa[:])
                # broadcast mean/rstd to channels: 4 matmuls into one psum
                # tile, then one batched a/b compute
                pabc = pstile([128, 8], F32, tag="sm")
                for t in range(CT):
                    nc.tensor.matmul(
                        pabc[:, 2 * t : 2 * t + 2],
                        selT_sb[:, t * 128 : (t + 1) * 128],
                        gs[:],
                        start=True,
                        stop=True,
                    )
                pabc_v = pabc[:].rearrange("p (t two) -> p t two", two=2)
                a4 = gnp.tile([128, 4], F32, tag="gn_A", name="gn_A")
                b4 = gnp.tile([128, 4], F32, tag="gn_B", name="gn_B")
                nc.vector.tensor_mul(a4[:], pabc_v[:, :, 1], gam_sb[:])
                nc.vector.tensor_mul(b4[:], pabc_v[:, :, 0], a4[:])
                nc.vector.tensor_tensor(b4[:], bet_sb[:], b4[:], op=SUB)
                for t in range(CT):
                    nc.vector.tensor_scalar(
                        dst_aps[t][:, 0:512],
                        src_tiles[t][:, 0:512],
                        scalar1=a4[:, t : t + 1],
                        scalar2=b4[:, t : t + 1],
                        op0=MULT,
                        op1=ADD,
                    )
                    nc.scalar.activation(
                        dst_aps[t][:, 512:1024],
                        src_tiles[t][:, 512:1024],
                        AF.Identity,
                        bias=b4[:, t : t + 1],
                        scale=a4[:, t : t + 1],
                    )

            # ---- GN1 -> xn (fp8 pair tiles for DoubleRow QKV) ----
            xnp = [
                pers.tile([128, 2, NSP], FP8, tag=f"xnp{p}", name=f"xnp{p}")
                for p in range(2)
            ]
            xn_aps = [xnp[t // 2][:, t % 2, :] for t in range(CT)]
            group_norm(x_sb, g1_sb, b1_sb, xn_aps)

            def dump_and_finish(aps, reuse=None, scale=None):
                # aps: list of 4 [128, NSP] APs; cast/scale to f32 and DMA out
                for t in range(CT):
                    ap = aps[t]
                    if reuse is not None:
                        ft = reuse[t]
                    else:
                        ft = pers.tile(
                            [128, NSP], F32, tag=f"dump{t}", name=f"dump{t}"
                        )
                    if scale is not None:
                        nc.vector.tensor_scalar_mul(ft[:], ap, scale)
                    else:
                        nc.vector.tensor_copy(ft[:], ap)
                    nc.sync.dma_start(out_d[t * 128 : (t + 1) * 128, :], ft[:])

            if KSTAGE == 1:
                dump_and_finish(xn_aps, scale=1.0)
                return nc

            # ---- QK (8 out tiles of 128 x 1024; fp8 DoubleRow, x64) ----
            qk = [
                pers.tile([128, NSP], BF16, tag=f"qk{m}", name=f"qk{m}")
                for m in range(8)
            ]
            # interleave q/k tiles so pair-j attention unblocks after 2j+2
            # readouts instead of 4+j
            for m in (0, 4, 1, 5, 2, 6, 3, 7):
                ps = pstile([128, NSP], F32)
                for p in range(2):
                    for n2 in range(2):
                        s = slice(n2 * 512, (n2 + 1) * 512)
                        nc.tensor.matmul(
                            ps[:, s],
                            wqk_sb[p][:, :, m * 128 : (m + 1) * 128],
                            xnp[p][:, :, s],
                            start=(p == 0),
                            stop=(p == 1),
                            perf_mode=DR,
                            skip_group_check=True,
                        )
                if m % 2 == 0:
                    nc.scalar.activation(
                        qk[m][:], ps[:], AF.Identity,
                        bias=qkb_sb[:, m : m + 1], scale=1.0 / WS,
                    )
                else:
                    nc.vector.tensor_scalar(
                        qk[m][:], ps[:],
                        scalar1=1.0 / WS, scalar2=qkb_sb[:, m : m + 1],
                        op0=MULT, op1=ADD,
                    )

            # ---- V^T directly: vt[key, d] = sum_c xn[c, key] * WS*Wv[d, c]
            # into fp8 pair tiles [128, 2, 640]: head h at cols 80h..80h+63,
            # ones column (denominator) at col 80h+64.
            vts = []
            for t in range(4):
                vt = pers.tile([128, 2, 640], EW, tag=f"vt{t}", name=f"vt{t}")
                nc.gpsimd.memset(vt[:].rearrange("p a b -> p (a b)"), 0.0)
                nc.gpsimd.memset(
                    vt[:].rearrange("p s (h c) -> p s h c", c=80)[:, :, :, 64:65],
                    1.0,
                )
                vts.append(vt)
            for mk in range(8):
                pv = pstile([128, 512], F32, tag="sm")
                for p in range(2):
                    nc.tensor.matmul(
                        pv[:],
                        xnp[p][:, :, mk * 128 : (mk + 1) * 128],
                        wv_sb[p][:, :, :],
                        start=(p == 0),
                        stop=(p == 1),
                        perf_mode=DR,
                        skip_group_check=True,
                    )
                dst = vts[mk // 2][:, mk % 2, :].rearrange(
                    "p (h d) -> p h d", h=8
                )[:, :, 0:64]
                src = pv[:].rearrange("p (h d) -> p h d", h=8)
                if mk % 2 == 0:
                    nc.vector.tensor_scalar_mul(dst, src, 1.0 / WS)
                else:
                    nc.scalar.activation(dst, src, AF.Identity, scale=1.0 / WS)

            if KSTAGE == 2:
                dump_and_finish([q[:] for q in qk[0:4]])
                return nc

            # ---- attention ----
            # pair j: head A = channels 0:64 of tile j, head B = 64:128.
            # QK both heads as concurrent row-tiled matmuls. exp written
            # straight into fp8 pair-layout AV-rhs tiles: side A on ACT
            # (real Exp, fp8 out), side B on DVE (schraudolph f32->int8).
            xap = [
                pers.tile([128, 2, NSP], FP8, tag=f"xap{p}", name=f"xap{p}")
                for p in range(2)
            ]

            def denom_chain(uns, j):
                # denominators: PE-transpose the two denom rows into
                # partitions, one parallel reciprocal, transpose back,
                # selector-matmul broadcast, normalize into fp8 pair tiles.
                # bf16 psum writes must stay 4B-aligned: put each denom
                # column at an even column index (stride-2 bf16)
                pdt = pstile([128, 32], BF16, tag="sm")
                pdt_v = pdt[:].rearrange("p (i two) -> p i two", two=2)
                for side in range(2):
                    for jj in range(8):
                        r = side * 8 + jj
                        nc.tensor.transpose(
                            pdt_v[:, r : r + 1, 0],
                            uns[side][64:65, jj * 128 : (jj + 1) * 128],
                            ones65[64:65, 0:1],
                        )
                inv16 = invp.tile([128, 16], F32, tag="inv", name="inv")
                nc.vector.reciprocal(inv16[:], pdt_v[:, :, 0])
                inv16b = invp.tile([128, 16], BF16, tag="invb16", name="invb16")
                nc.vector.tensor_copy(inv16b[:], inv16[:])
                ptv = pstile([16, 128], BF16, tag="sm")
                nc.tensor.transpose(ptv[:], inv16b[:], id_sb[:])
                pts = invp.tile([16, 128], BF16, tag="pts", name="pts")
                nc.vector.tensor_copy(pts[:], ptv[:])
                for side in range(2):
                    for n2 in range(2):
                        pinvb = pstile([64, 512], F32, tag="sm")
                        for jj in range(4):
                            r = side * 8 + n2 * 4 + jj
                            nc.tensor.matmul(
                                pinvb[:, jj * 128 : (jj + 1) * 128],
                                selbc_sb[:, r * 64 : (r + 1) * 64],
                                pts[:],
                                start=True,
                                stop=True,
                            )
                        nc.vector.tensor_mul(
                            xap[j // 2][
                                64 * side : 64 * side + 64,
                                j % 2,
                                n2 * 512 : (n2 + 1) * 512,
                            ],
                            uns[side][0:64, n2 * 512 : (n2 + 1) * 512],
                            pinvb[:],
                        )

            for j in range(4):
                # exp-weight tiles for this pair: per (side, t):
                # fp8 [128, 2(s), 1024]; or f32 z-tiles for bf16 fallback
                if EXPBF:
                    ex = [
                        [
                            exq.tile(
                                [128, 2, 2, 512], F32,
                                tag=f"ex{side}{t}", name=f"ex{side}{t}", bufs=1,
                            )
                            for t in range(4)
                        ]
                        for side in range(2)
                    ]
                else:
                    ex = [
                        [
                            exq.tile(
                                [128, 2, NSP], FP8,
                                tag=f"ex{side}{t}", name=f"ex{side}{t}",
                            )
                            for t in range(4)
                        ]
                        for side in range(2)
                    ]
                uns = [
                    unp.tile([65, NSP], BF16, tag=f"un{side}", name=f"un{side}")
                    for side in range(2)
                ]

                def av_pass(n2):
                    for side in range(2):
                        hh = 2 * j + side
                        pav = pstile([66, 512], F32, tag="sm")
                        if EXPBF:
                            for t in range(4):
                                for s2 in range(2):
                                    rhs = (
                                        ex[side][t][:]
                                        .bitcast(BF16)
                                        .rearrange(
                                            "p a b (n two) -> p a b n two", two=2
                                        )[:, s2, n2, :, 0]
                                    )
                                    nc.tensor.matmul(
                                        pav[:],
                                        vts[t][:, s2, 80 * hh : 80 * hh + 66],
                                        rhs,
                                        start=(t == 0 and s2 == 0),
                                        stop=(t == 3 and s2 == 1),
                                        skip_group_check=True,
                                    )
                        else:
                            for t in range(4):
                                nc.tensor.matmul(
                                    pav[:],
                                    vts[t][:, :, 80 * hh : 80 * hh + 66],
                                    ex[side][t][:, :, n2 * 512 : (n2 + 1) * 512],
                                    start=(t == 0),
                                    stop=(t == 3),
                                    perf_mode=DR,
                                    skip_group_check=True,
                                )
                        nc.scalar.activation(
                            uns[side][:, n2 * 512 : (n2 + 1) * 512],
                            pav[0:65, :],
                            AF.Identity,
                            scale=1.0,
                        )

                for t in range(4):
                    for s2 in range(2):
                        mk = 2 * t + s2
                        mks = slice(mk * 128, (mk + 1) * 128)
                        scs = []
                        for side in range(2):
                            ph = slice(64 * side, 64 * side + 64)
                            sc = pstile([128, NSP], F32, tag="ps")
                            for n2 in range(2):
                                s = slice(n2 * 512, (n2 + 1) * 512)
                                nc.tensor.matmul(
                                    sc[:, s],
                                    qk[4 + j][ph, mks],
                                    qk[j][ph, s],
                                    start=True,
                                    stop=True,
                                )
                            scs.append(sc)
                        # exp: side A on ACT (fp8 direct), side B on DVE
                        # (schraudolph bits via f32->int8 round)
                        if EXPBF:
                            nc.scalar.activation(
                                ex[0][t][:, s2, :, :].rearrange("p a b -> p (a b)")
                                .bitcast(BF16)
                                .rearrange("p (n two) -> p n two", two=2)[:, :, 0],
                                scs[0][:],
                                AF.Exp,
                                scale=0.125,
                            )
                            nc.vector.tensor_scalar(
                                ex[1][t][:, s2, :, :].rearrange(
                                    "p a b -> p (a b)"
                                ),
                                scs[1][:],
                                scalar1=EXPA16,
                                scalar2=EXPB16,
                                op0=MULT,
                                op1=ADD,
                            )
                        else:
                            nc.scalar.activation(
                                ex[0][t][:, s2, :], scs[0][:], AF.Exp, scale=0.125
                            )
                            nc.vector.tensor_scalar(
                                ex[1][t][:, s2, :].bitcast(I8),
                                scs[1][:],
                                scalar1=EXPA8,
                                scalar2=EXPB8,
                                op0=MULT,
                                op1=ADD,
                            )
                # AV passes: n2=0 then n2=1 (ex tiles persist in SBUF)
                av_pass(0)
                av_pass(1)
                denom_chain(uns, j)

            if KSTAGE == 3:
                dump_and_finish([xap[t // 2][:, t % 2, :] for t in range(CT)])
                return nc

            load_late_weights()

            # ---- out projection (fp8 DoubleRow; keep f32 for GN2 stats) ----
            attn2 = [
                pers.tile([128, NSP], F32, tag=f"attn2{t}", name=f"attn2{t}")
                for t in range(CT)
            ]
            for m in range(CT):
                ps = pstile([128, NSP], F32)
                for p in range(2):
                    for n2 in range(2):
                        s = slice(n2 * 512, (n2 + 1) * 512)
                        nc.tensor.matmul(
                            ps[:, s],
                            wo_sb[p][:, :, m * 128 : (m + 1) * 128],
                            xap[p][:, :, s],
                            start=(p == 0),
                            stop=(p == 1),
                            perf_mode=DR,
                            skip_group_check=True,
                        )
                nc.scalar.activation(
                    attn2[m][:], ps[:], AF.Identity,
                    bias=outb_sb[:, m : m + 1], scale=1.0 / WS,
                )

            if KSTAGE == 4:
                for t in range(CT):
                    nc.sync.dma_start(out_d[t * 128 : (t + 1) * 128, :], attn2[t][:])
                return nc

            # ---- GN2 -> xg (fp8 pair tiles) ----
            xgp = [
                pers.tile([128, 2, NSP], FP8, tag=f"xgp{p}", name=f"xgp{p}")
                for p in range(2)
            ]
            xg_aps = [xgp[t // 2][:, t % 2, :] for t in range(CT)]
            group_norm(attn2, g2_sb, b2_sb, xg_aps)

            if KSTAGE == 5:
                dump_and_finish(xg_aps, reuse=attn2, scale=1.0)
                return nc

            # ---- MLP1 + SwiGLU (fp8 DoubleRow; h1 x64 descaled in Silu's
            # scale, gate x4 compensated in the MLP2 readout) ----
            actp = [
                pers.tile([128, 2, NSP], FP8, tag=f"actp{p}", name=f"actp{p}")
                for p in range(8)
            ]
            for mp in range(16):
                ps1 = pstile([128, NSP], F32)
                for p in range(2):
                    for n2 in range(2):
                        s = slice(n2 * 512, (n2 + 1) * 512)
                        nc.tensor.matmul(
                            ps1[:, s],
                            w1_sb[p][:, :, mp * 128 : (mp + 1) * 128],
                            xgp[p][:, :, s],
                            start=(p == 0),
                            stop=(p == 1),
                            perf_mode=DR,
                            skip_group_check=True,
                        )
                ps2 = pstile([128, NSP], F32)
                for p in range(2):
                    for n2 in range(2):
                        s = slice(n2 * 512, (n2 + 1) * 512)
                        nc.tensor.matmul(
                            ps2[:, s],
                            w1_sb[p][:, :, (mp + 16) * 128 : (mp + 17) * 128],
                            xgp[p][:, :, s],
                            start=(p == 0),
                            stop=(p == 1),
                            perf_mode=DR,
                            skip_group_check=True,
                        )
                sg = swp.tile([128, NSP], BF16, tag="sw", name="sw")
                nc.scalar.activation(sg[:], ps1[:], AF.Silu, scale=1.0 / WS)
                nc.vector.tensor_mul(actp[mp // 2][:, mp % 2, :], sg[:], ps2[:])

            if KSTAGE == 6:
                dump_and_finish(
                    [actp[t // 2][:, t % 2, :] for t in range(CT)],
                    reuse=attn2, scale=1.0 / GS,
                )
                return nc

            # reload x into the attn2 slots (attention result consumed by GN2)
            for t in range(CT):
                nc.sync.dma_start(attn2[t][:], x_d[t * 128 : (t + 1) * 128, :])

            # ---- MLP2 + residual -> out (fp8 DoubleRow) ----
            for m in range(CT):
                ps = pstile([128, NSP], F32)
                for p in range(8):
                    for n2 in range(2):
                        s = slice(n2 * 512, (n2 + 1) * 512)
                        nc.tensor.matmul(
                            ps[:, s],
                            w2_sb[p][:, :, m * 128 : (m + 1) * 128],
                            actp[p][:, :, s],
                            start=(p == 0),
                            stop=(p == 7),
                            perf_mode=DR,
                            skip_group_check=True,
                        )
                tmp = swp.tile([128, NSP], F32, tag="mlp2t", name="mlp2t")
                nc.scalar.activation(tmp[:], ps[:], AF.Copy, scale=1.0 / (WS * GS))
                nc.vector.tensor_tensor(attn2[m][:], tmp[:], attn2[m][:], op=ADD)
                nc.sync.dma_start(out_d[m * 128 : (m + 1) * 128, :], attn2[m][:])

    return nc


def _get_nc():
    key = ("nc", KSTAGE, EXPBF)
    if key not in _cache:
        import concourse.bass  # noqa: F401  ensure importable before build
        from concourse import mybir

        res = _build_nc()
        nc = res[0] if isinstance(res, tuple) else res
        _split_multi_waits(nc, mybir, maxw=1)
        _cache[key] = nc
    return _cache[key]


def _pair_pack(wT, pairs):
    """wT: [K, M] -> list of `pairs` arrays [128, 2*M] with
    [q, 2s+m]... = wT[256p+128s+q, m] laid out [128][2][M] contiguously."""
    K, M = wT.shape
    assert K == pairs * 256
    out = []
    for p in range(pairs):
        blk = wT[p * 256 : (p + 1) * 256].reshape(2, 128, M).transpose(1, 0, 2)
        out.append(np.ascontiguousarray(blk.reshape(128, 2 * M)))
    return out


def _prep_weights(inputs):
    bf = ml_dtypes.bfloat16
    f8 = ml_dtypes.float8_e4m3
    f32 = np.float32

    def col4(v):  # (512,) -> (128, 4) with [p, t] = v[128t + p]
        return np.ascontiguousarray(v.reshape(4, 128).T.astype(f32))

    qkv_b = inputs["qkv_b"].astype(f32)
    sel = np.zeros((C, 32), f32)
    sel[np.arange(C), np.arange(C) // 16] = 1.0
    selbc = np.zeros((16, 1024), f32)
    for r in range(16):
        selbc[r, r * 64 : (r + 1) * 64] = 1.0
    selbc = selbc.astype(bf)

    wqkvT = np.ascontiguousarray(inputs["qkv_w"].astype(f32).T)
    wqkT = wqkvT[:, 0 : 2 * C] * WS
    wvT = wqkvT[:, 2 * C :] * WS
    woT = np.ascontiguousarray(inputs["out_w"].astype(f32).T) * WS
    w1T = np.ascontiguousarray(inputs["mlp1_w"].astype(f32).T).copy()
    w1T[:, 0:HID] *= WS
    w1T[:, HID:] *= GS
    w2T = np.ascontiguousarray(inputs["mlp2_w"].astype(f32).T) * WS

    # v bias folded into out-proj bias (softmax rows sum to 1)
    ob = inputs["out_b"].astype(f32) + inputs["out_w"].astype(f32) @ qkv_b[2 * C :]

    shared = {
        "qkb": np.ascontiguousarray(qkv_b[0 : 2 * C].reshape(8, 128).T.astype(f32)),
        "outb": col4(ob),
        "g1": col4(inputs["gn1_gamma"].astype(f32)),
        "b1": col4(inputs["gn1_beta"].astype(f32)),
        "g2": col4(inputs["gn2_gamma"].astype(f32)),
        "b2": col4(inputs["gn2_beta"].astype(f32)),
        "sel": sel,
        "selT": np.ascontiguousarray(sel.T),
        "ident": np.eye(128, dtype=f32).astype(bf),
        "selbc": selbc,
    }
    for p, a in enumerate(_pair_pack(wqkT, 2)):
        shared[f"wqkT{p}"] = a.astype(f8)
    for p, a in enumerate(_pair_pack(wvT, 2)):
        shared[f"wvT{p}"] = a.astype(f8)
    for p, a in enumerate(_pair_pack(woT, 2)):
        shared[f"woT{p}"] = a.astype(f8)
    for p, a in enumerate(_pair_pack(w1T, 2)):
        shared[f"w1T{p}"] = a.astype(f8)
    for p, a in enumerate(_pair_pack(w2T, 8)):
        shared[f"w2T{p}"] = a.astype(f8)
    return shared


def kernel(**inputs):
    from concourse.bass_utils import run_bass_kernel_spmd

    nc = _get_nc()
    shared = _prep_weights(inputs)
    x = np.asarray(inputs["x"], dtype=np.float32).reshape(8, C, NSP)
    in_maps = [dict(shared, x=np.ascontiguousarray(x[i])) for i in range(8)]
    res = run_bass_kernel_spmd(nc, in_maps, core_ids=list(range(8))).results
    out = np.stack([res[i]["out"] for i in range(8)], axis=0)
    return out.reshape(8, C, 32, 32).astype(np.float32)


# revision 25
# speedup vs baseline: 1.0182x; 1.0182x over previous
"""Trainium2 Bass kernel for nn_Attention_Block (dense transformer block).

Strategy: pure data-parallel over batch -- 8 samples, 8 NeuronCores, one
sample per core, weights replicated, no collectives. Per core everything
stays channels-on-partitions (c x n layout):

  GN1 (bn_stats + selector-matmul group reduce) -> QKV matmul for q,k (fp8
  DoubleRow, weights x64) -> V^T computed directly as xn^T @ Wv^T (fp8
  DoubleRow, no PE transposes) into fp8 pair-layout tiles with a ones
  column per head for the softmax denominator -> per-head-pair attention:
  QK row-tiled (both heads concurrent on disjoint 64-partition groups),
  exp emitted directly as fp8 e4m3 in DoubleRow pair layout (side A on
  ACT via real Exp with fp8 output; side B on DVE via a Schraudolph
  bit-trick: z = score*log2e + (56 + 8c), round-to-nearest f32->int8
  convert, bits reinterpreted as e4m3), AV as fp8 DoubleRow matmuls
  contracting 256 keys per instruction -> denominator chain (PE column
  transposes -> one parallel reciprocal -> broadcast matmul) -> out-proj
  (fp8 DoubleRow) -> GN2 -> SwiGLU MLP (fp8 DoubleRow both matmuls) ->
  +residual.

Weight matrices are pre-scaled x64 into fp8 e4m3 (descale folded into the
PSUM readout ops); the SwiGLU gate half is scaled x4 with the
compensating 1/4 folded into the MLP2 readout. V's qkv bias is folded
into the out-proj bias host-side (softmax rows sum to 1).
"""

import os

import numpy as np
import ml_dtypes

KSTAGE = int(os.environ.get("KSTAGE", "7"))
EXPBF = int(os.environ.get("EXPBF", "0"))  # 1 = bf16 exp weights fallback

C = 512
NSP = 1024  # 32*32 spatial
CT = 4  # channel tiles of 128
HEADS = 8
D = 64
HID = 2048
EPS = 1e-5
WS = 64.0  # fp8 weight scale
GS = 4.0  # extra scale on the SwiGLU gate half
LOG2E = 1.4426950408889634
# fp8(e4m3) schraudolph: bits = round(score*log2e + 8*(7 + c)), c calibrated
CEXP8 = 0.095
EXPA8 = LOG2E
EXPB8 = 8.0 * (7.0 + CEXP8)
# bf16 fallback schraudolph (baseline constants)
CEXP = -0.0505
EXPA16 = 0.125 * 128.0 * LOG2E
EXPB16 = 128.0 * (127.0 + CEXP) + 8388608.0

_cache = {}


def _patch_tile_drain(tile, mybir):
    """walrus in this environment accepts very few sync waits per
    instruction; the TileContext tail drain carries one wait per proc of
    the global clock. Split them across preceding SP drains."""
    if getattr(tile.TileContext, "_drain_patched", False):
        return

    def _patched(self, tick_clock, wait_clock):
        nc = self.nc
        spills = [nc.sync.drain() for _ in range(40)]
        drain_inst = nc.sync.drain()
        wait_clock.add_sem_waits(
            drain_inst.ins, tile.ScopedClock({None: tick_clock.global_clock})
        )
        si = drain_inst.ins.sync_info
        waits = list(si.on_wait) if si is not None and si.on_wait else []
        upds = list(si.on_update) if si is not None and si.on_update else []
        if len(waits) > 1:
            *pre, last = waits
            assert len(pre) <= len(spills), "too many drain wait chunks"
            for sp_inst, w in zip(spills, pre):
                sp_inst.ins.sync_info = mybir.SyncInfo(on_wait=[w], on_update=[])
            drain_inst.ins.sync_info = mybir.SyncInfo(on_wait=[last], on_update=upds)
        nc.all_engine_barrier()
        assert self.sems is not None
        popped = nc._tile_sem_poison_stack.pop()
        assert popped is self._sem_poison
        nc.clear_and_free_semaphores(list(self.sems.allocated().values()))
        nc.all_engine_barrier()

    tile.TileContext._drain_and_barrier = _patched
    tile.TileContext._drain_patched = True


def _split_multi_waits(nc, mybir, maxw=1):
    """Hoist extra sync waits onto same-engine EventSemaphore carriers so
    no instruction carries more than `maxw` waits."""
    f = nc.m.functions[0]
    for bb in f.blocks:
        insts = list(bb.instructions)
        need = [
            i
            for i in insts
            if getattr(i, "sync_info", None)
            and i.sync_info.on_wait
            and len(i.sync_info.on_wait) > maxw
        ]
        if not need:
            continue
        carriers = {}
        for inst in need:
            w = list(inst.sync_info.on_wait)
            upds = list(inst.sync_info.on_update) if inst.sync_info.on_update else []
            keep = w[-maxw:]
            extra = w[:-maxw]
            cs = []
            for i in range(0, len(extra), maxw):
                c = mybir.InstEventSemaphore(
                    name=f"I-waitc-{nc.next_id()}", ins=[], outs=[]
                )
                c.engine = inst.engine
                c.sync_info = mybir.SyncInfo(on_wait=extra[i : i + maxw], on_update=[])
                nc.register_instruction(c)
                cs.append(c)
            inst.sync_info = mybir.SyncInfo(on_wait=keep, on_update=upds)
            carriers[inst.name] = cs
        carrier_names = {c.name for cs in carriers.values() for c in cs}
        rebuilt = []
        for inst in list(bb.instructions):
            if inst.name in carrier_names:
                continue
            if inst.name in carriers:
                rebuilt.extend(carriers[inst.name])
            rebuilt.append(inst)
        bb.instructions = rebuilt


def _build_nc():
    import concourse.bass as bass
    import concourse.tile as tile
    from concourse import mybir

    _patch_tile_drain(tile, mybir)

    F32 = mybir.dt.float32
    BF16 = mybir.dt.bfloat16
    FP8 = mybir.dt.float8e4
    I8 = mybir.dt.int8
    ADD = mybir.AluOpType.add
    SUB = mybir.AluOpType.subtract
    MULT = mybir.AluOpType.mult
    AF = mybir.ActivationFunctionType
    DR = mybir.MatmulPerfMode.DoubleRow

    nc = bass.Bass()

    x_d = nc.declare_dram_parameter("x", [C, NSP], F32, isOutput=False)
    # fp8 pair-layout weights: [128, 2*cols]; [q, 2s+m] = W.T[256p+128s+q, m]
    wqk_d = [
        nc.declare_dram_parameter(f"wqkT{p}", [128, 2 * 2 * C], FP8, isOutput=False)
        for p in range(2)
    ]
    wv_d = [
        nc.declare_dram_parameter(f"wvT{p}", [128, 2 * C], FP8, isOutput=False)
        for p in range(2)
    ]
    qkb_d = nc.declare_dram_parameter("qkb", [128, 8], F32, isOutput=False)
    wo_d = [
        nc.declare_dram_parameter(f"woT{p}", [128, 2 * C], FP8, isOutput=False)
        for p in range(2)
    ]
    outb_d = nc.declare_dram_parameter("outb", [128, 4], F32, isOutput=False)
    g1_d = nc.declare_dram_parameter("g1", [128, 4], F32, isOutput=False)
    b1_d = nc.declare_dram_parameter("b1", [128, 4], F32, isOutput=False)
    g2_d = nc.declare_dram_parameter("g2", [128, 4], F32, isOutput=False)
    b2_d = nc.declare_dram_parameter("b2", [128, 4], F32, isOutput=False)
    w1_d = [
        nc.declare_dram_parameter(f"w1T{p}", [128, 2 * 2 * HID], FP8, isOutput=False)
        for p in range(2)
    ]
    w2_d = [
        nc.declare_dram_parameter(f"w2T{p}", [128, 2 * C], FP8, isOutput=False)
        for p in range(8)
    ]
    sel_d = nc.declare_dram_parameter("sel", [C, 32], BF16, isOutput=False)
    selT_d = nc.declare_dram_parameter("selT", [32, C], BF16, isOutput=False)
    id_d = nc.declare_dram_parameter("ident", [128, 128], BF16, isOutput=False)
    selbc_d = nc.declare_dram_parameter("selbc", [16, 1024], BF16, isOutput=False)
    out_d = nc.declare_dram_parameter("out", [C, NSP], F32, isOutput=True)

    EW = BF16 if EXPBF else FP8  # exp-weight / V dtype

    with tile.TileContext(nc) as tc:
        with (
            tc.tile_pool(name="pers", bufs=1) as pers,
            tc.tile_pool(name="gnp", bufs=2) as gnp,
            tc.tile_pool(name="exq", bufs=2) as exq,
            tc.tile_pool(name="unp", bufs=2) as unp,
            tc.tile_pool(name="invp", bufs=2) as invp,
            tc.tile_pool(name="swp", bufs=2) as swp,
            tc.tile_pool(name="ps", bufs=2, space="PSUM") as ps_pool,
        ):
            # PSUM budget (8 banks): tag "ps" = 2-bank slots x3 (big f32
            # [128,1024] tiles: QKV/MLP/out-proj psums + attention score
            # tiles -- 3 slots so QK of round r+1 overlaps exp of round
            # r), tag "sm" = 1-bank x2 (av/pv/pdt/ptv/pinvb/gn).
            def pstile(shape, dtype, tag="ps", bufs=None):
                if bufs is None:
                    bufs = 3 if tag == "ps" else 2
                return ps_pool.tile(shape, dtype, tag=tag, name=tag, bufs=bufs)

            # ---- input loads (x lands in the attn2 slots; reloaded later) ----
            x_sb = []
            x_engs = [nc.sync, nc.scalar, nc.sync, nc.scalar]
            for t in range(CT):
                xt = pers.tile([128, NSP], F32, tag=f"attn2{t}", name=f"attn2{t}")
                x_engs[t].dma_start(xt[:], x_d[t * 128 : (t + 1) * 128, :])
                x_sb.append(xt)
            sel_sb = []
            for t in range(CT):
                st = pers.tile([128, 32], BF16, tag=f"sel{t}", name=f"sel{t}")
                nc.sync.dma_start(st[:], sel_d[t * 128 : (t + 1) * 128, :])
                sel_sb.append(st)
            selT_sb = pers.tile([32, C], BF16, tag="selT", name="selT")
            nc.sync.dma_start(selT_sb[:], selT_d[:, :])
            g1_sb = pers.tile([128, 4], F32, tag="g1", name="g1")
            nc.sync.dma_start(g1_sb[:], g1_d[:, :])
            b1_sb = pers.tile([128, 4], F32, tag="b1", name="b1")
            nc.sync.dma_start(b1_sb[:], b1_d[:, :])
            wqk_sb = []
            for p in range(2):
                wt = pers.tile([128, 2, 2 * C], FP8, tag=f"wqk{p}", name=f"wqk{p}")
                nc.sync.dma_start(wt[:].rearrange("p a b -> p (a b)"), wqk_d[p][:, :])
                wqk_sb.append(wt)
            wv_sb = []
            for p in range(2):
                wt = pers.tile([128, 2, C], FP8, tag=f"wv{p}", name=f"wv{p}")
                nc.scalar.dma_start(wt[:].rearrange("p a b -> p (a b)"), wv_d[p][:, :])
                wv_sb.append(wt)
            qkb_sb = pers.tile([128, 8], F32, tag="qkb", name="qkb")
            nc.sync.dma_start(qkb_sb[:], qkb_d[:, :])
            selbc_sb = pers.tile([16, 1024], BF16, tag="selbc", name="selbc")
            nc.sync.dma_start(selbc_sb[:], selbc_d[:, :])
            id_sb = pers.tile([128, 128], BF16, tag="ident", name="ident")
            nc.sync.dma_start(id_sb[:], id_d[:, :])
            wo_sb = [
                pers.tile([128, 2, C], FP8, tag=f"wo{p}", name=f"wo{p}")
                for p in range(2)
            ]
            outb_sb = pers.tile([128, 4], F32, tag="outb", name="outb")
            g2_sb = pers.tile([128, 4], F32, tag="g2", name="g2")
            b2_sb = pers.tile([128, 4], F32, tag="b2", name="b2")
            w1_sb = [
                pers.tile([128, 2, 2 * HID], FP8, tag=f"w1{p}", name=f"w1{p}")
                for p in range(2)
            ]
            w2_sb = [
                pers.tile([128, 2, C], FP8, tag=f"w2{p}", name=f"w2{p}")
                for p in range(8)
            ]

            def load_late_weights():
                # issued after attention emission: keeps early HBM
                # bandwidth for x and the attention-phase inputs
                for p in range(2):
                    nc.sync.dma_start(
                        wo_sb[p][:].rearrange("p a b -> p (a b)"), wo_d[p][:, :]
                    )
                nc.sync.dma_start(outb_sb[:], outb_d[:, :])
                nc.sync.dma_start(g2_sb[:], g2_d[:, :])
                nc.sync.dma_start(b2_sb[:], b2_d[:, :])
                for p in range(2):
                    nc.sync.dma_start(
                        w1_sb[p][:].rearrange("p a b -> p (a b)"), w1_d[p][:, :]
                    )
                for p in range(8):
                    nc.sync.dma_start(
                        w2_sb[p][:].rearrange("p a b -> p (a b)"), w2_d[p][:, :]
                    )

            eps32 = pers.tile([32, 1], F32, tag="eps", name="eps")
            nc.vector.memset(eps32[:], EPS)
            warm = pers.tile([1, 1], F32, tag="warm", name="warm")
            nc.vector.memset(warm[:], 1.0)
            nc.scalar.activation(warm[:], warm[:], AF.Exp)
            magic = pers.tile([32, 1], mybir.dt.int32, tag="magic", name="magic")
            nc.vector.memset(magic[:], 1597463007)
            ones65 = pers.tile([65, 1], BF16, tag="ones65", name="ones65")
            nc.vector.memset(ones65[:], 1.0)

            # ---- group norm helper (32 groups of 16 channels x 1024) ----
            def group_norm(src_tiles, gam_sb, bet_sb, dst_aps):
                rhs3 = []
                for t in range(CT):
                    stats = gnp.tile([128, 2, 6], F32, tag="gn_stats", name="gn_stats")
                    for j2 in range(2):
                        nc.vector.bn_stats(
                            stats[:, j2, :], src_tiles[t][:, j2 * 512 : (j2 + 1) * 512]
                        )
                    mv = gnp.tile([128, 2], F32, tag="gn_mv", name="gn_mv")
                    nc.vector.bn_aggr(mv[:], stats[:])
                    r3 = gnp.tile([128, 3], BF16, tag=f"gn_r3_{t}", name=f"gn_r3_{t}")
                    nc.vector.tensor_copy(r3[:, 0:2], mv[:])
                    nc.vector.tensor_mul(r3[:, 2:3], mv[:, 0:1], mv[:, 0:1])
                    rhs3.append(r3)
                pg = pstile([32, 3], F32, tag="sm")
                for t in range(CT):
                    nc.tensor.matmul(
                        pg[:], sel_sb[t][:], rhs3[t][:], start=(t == 0), stop=(t == 3)
                    )
                gs = gnp.tile([32, 2], F32, tag="gn_gs", name="gn_gs")
                tmp = gnp.tile([32, 2], F32, tag="gn_tmp", name="gn_tmp")
                pgs = gnp.tile([32, 3], F32, tag="gn_pgs", name="gn_pgs")
                nc.vector.tensor_copy(pgs[:], pg[:])
                # mean_g, E[x^2]_g, var_g, rstd_g
                nc.vector.tensor_scalar_mul(gs[:, 0:1], pgs[:, 0:1], 1.0 / 16)
                nc.vector.tensor_tensor(tmp[:, 0:1], pgs[:, 1:2], pgs[:, 2:3], op=ADD)
                nc.vector.tensor_scalar_mul(tmp[:, 0:1], tmp[:, 0:1], 1.0 / 16)
                nc.vector.tensor_mul(tmp[:, 1:2], gs[:, 0:1], gs[:, 0:1])
                nc.vector.tensor_tensor(tmp[:, 0:1], tmp[:, 0:1], tmp[:, 1:2], op=SUB)
                # rstd via quake rsqrt + 1 Newton step (max rel err ~0.2%)
                ve = gnp.tile([32, 1], F32, tag="gn_ve", name="gn_ve")
                nc.vector.tensor_scalar_add(ve[:], tmp[:, 0:1], EPS)
                ve2 = gnp.tile([32, 1], F32, tag="gn_ve2", name="gn_ve2")
                nc.vector.tensor_scalar_mul(ve2[:], ve[:], 0.5)
                yb = gnp.tile([32, 1], mybir.dt.int32, tag="gn_yb", name="gn_yb")
                nc.vector.tensor_scalar(
                    yb[:], ve[:].bitcast(mybir.dt.int32),
                    scalar1=1, scalar2=None,
                    op0=mybir.AluOpType.logical_shift_right,
                )
                y0 = gnp.tile([32, 1], mybir.dt.int32, tag="gn_y0", name="gn_y0")
                nc.vector.tensor_tensor(y0[:], magic[:], yb[:], op=SUB)
                ycur = y0[:].bitcast(F32)
                aa = gnp.tile([32, 1], F32, tag="gn_a0", name="gn_a0")
                nc.vector.tensor_mul(aa[:], ycur, ycur)
                nc.vector.tensor_mul(aa[:], aa[:], ve2[:])
                nc.vector.tensor_scalar(
                    aa[:], aa[:], scalar1=-1.0, scalar2=1.5, op0=MULT, op1=ADD
                )
                nc.vector.tensor_mul(gs[:, 1:2], ycur, aa[:])
                # broadcast mean/rstd to channels: 4 bf16 matmuls into one
                # psum tile, then one batched a/b compute
                gsb = gnp.tile([32, 2], BF16, tag="gn_gsb", name="gn_gsb")
                nc.vector.tensor_copy(gsb[:], gs[:])
                pabc = pstile([128, 8], F32, tag="sm")
                for t in range(CT):
                    nc.tensor.matmul(
                        pabc[:, 2 * t : 2 * t + 2],
                        selT_sb[:, t * 128 : (t + 1) * 128],
                        gsb[:],
                        start=True,
                        stop=True,
                    )
                pabc_v = pabc[:].rearrange("p (t two) -> p t two", two=2)
                a4 = gnp.tile([128, 4], F32, tag="gn_A", name="gn_A")
                b4 = gnp.tile([128, 4], F32, tag="gn_B", name="gn_B")
                nc.vector.tensor_mul(a4[:], pabc_v[:, :, 1], gam_sb[:])
                nc.vector.tensor_mul(b4[:], pabc_v[:, :, 0], a4[:])
                nc.vector.tensor_tensor(b4[:], bet_sb[:], b4[:], op=SUB)
                for t in range(CT):
                    nc.vector.tensor_scalar(
                        dst_aps[t][:, 0:512],
                        src_tiles[t][:, 0:512],
                        scalar1=a4[:, t : t + 1],
                        scalar2=b4[:, t : t + 1],
                        op0=MULT,
                        op1=ADD,
                    )
                    nc.scalar.activation(
                        dst_aps[t][:, 512:1024],
                        src_tiles[t][:, 512:1024],
                        AF.Identity,
                        bias=b4[:, t : t + 1],
                        scale=a4[:, t : t + 1],
                    )

            # ---- GN1 -> xn (fp8 pair tiles for DoubleRow QKV) ----
            xnp = [
                pers.tile([128, 2, NSP], FP8, tag=f"xnp{p}", name=f"xnp{p}")
                for p in range(2)
            ]
            xn_aps = [xnp[t // 2][:, t % 2, :] for t in range(CT)]
            group_norm(x_sb, g1_sb, b1_sb, xn_aps)

            def dump_and_finish(aps, reuse=None, scale=None):
                # aps: list of 4 [128, NSP] APs; cast/scale to f32 and DMA out
                for t in range(CT):
                    ap = aps[t]
                    if reuse is not None:
                        ft = reuse[t]
                    else:
                        ft = pers.tile(
                            [128, NSP], F32, tag=f"dump{t}", name=f"dump{t}"
                        )
                    if scale is not None:
                        nc.vector.tensor_scalar_mul(ft[:], ap, scale)
                    else:
                        nc.vector.tensor_copy(ft[:], ap)
                    nc.sync.dma_start(out_d[t * 128 : (t + 1) * 128, :], ft[:])

            if KSTAGE == 1:
                dump_and_finish(xn_aps, scale=1.0)
                return nc

            # ---- QK (8 out tiles of 128 x 1024; fp8 DoubleRow, x64) ----
            qk = [
                pers.tile([128, NSP], BF16, tag=f"qk{m}", name=f"qk{m}")
                for m in range(8)
            ]
            # interleave q/k tiles so pair-j attention unblocks after 2j+2
            # readouts instead of 4+j
            for m in (0, 4, 1, 5, 2, 6, 3, 7):
                ps = pstile([128, NSP], F32)
                for p in range(2):
                    for n2 in range(2):
                        s = slice(n2 * 512, (n2 + 1) * 512)
                        nc.tensor.matmul(
                            ps[:, s],
                            wqk_sb[p][:, :, m * 128 : (m + 1) * 128],
                            xnp[p][:, :, s],
                            start=(p == 0),
                            stop=(p == 1),
                            perf_mode=DR,
                            skip_group_check=True,
                        )
                if m % 2 == 0:
                    nc.scalar.activation(
                        qk[m][:], ps[:], AF.Identity,
                        bias=qkb_sb[:, m : m + 1], scale=1.0 / WS,
                    )
                else:
                    nc.vector.tensor_scalar(
                        qk[m][:], ps[:],
                        scalar1=1.0 / WS, scalar2=qkb_sb[:, m : m + 1],
                        op0=MULT, op1=ADD,
                    )

            # ---- V^T directly: vt[key, d] = sum_c xn[c, key] * WS*Wv[d, c]
            # into fp8 pair tiles [128, 2, 640]: head h at cols 80h..80h+63,
            # ones column (denominator) at col 80h+64.
            vts = []
            for t in range(4):
                vt = pers.tile([128, 2, 640], EW, tag=f"vt{t}", name=f"vt{t}")
                nc.gpsimd.memset(vt[:].rearrange("p a b -> p (a b)"), 0.0)
                nc.gpsimd.memset(
                    vt[:].rearrange("p s (h c) -> p s h c", c=80)[:, :, :, 64:65],
                    1.0,
                )
                vts.append(vt)
            for mk in range(8):
                pv = pstile([128, 512], F32, tag="sm")
                for p in range(2):
                    nc.tensor.matmul(
                        pv[:],
                        xnp[p][:, :, mk * 128 : (mk + 1) * 128],
                        wv_sb[p][:, :, :],
                        start=(p == 0),
                        stop=(p == 1),
                        perf_mode=DR,
                        skip_group_check=True,
                    )
                dst = vts[mk // 2][:, mk % 2, :].rearrange(
                    "p (h d) -> p h d", h=8
                )[:, :, 0:64]
                src = pv[:].rearrange("p (h d) -> p h d", h=8)
                if mk % 2 == 0:
                    nc.vector.tensor_scalar_mul(dst, src, 1.0 / WS)
                else:
                    nc.scalar.activation(dst, src, AF.Identity, scale=1.0 / WS)

            if KSTAGE == 2:
                dump_and_finish([q[:] for q in qk[0:4]])
                return nc

            # ---- attention ----
            # pair j: head A = channels 0:64 of tile j, head B = 64:128.
            # QK both heads as concurrent row-tiled matmuls. exp written
            # straight into fp8 pair-layout AV-rhs tiles: side A on ACT
            # (real Exp, fp8 out), side B on DVE (schraudolph f32->int8).
            xap = [
                pers.tile([128, 2, NSP], FP8, tag=f"xap{p}", name=f"xap{p}")
                for p in range(2)
            ]

            def denom_chain(uns, j):
                # denominators: PE-transpose the two denom rows into
                # partitions, one parallel reciprocal, transpose back,
                # selector-matmul broadcast, normalize into fp8 pair tiles.
                # bf16 psum writes must stay 4B-aligned: put each denom
                # column at an even column index (stride-2 bf16)
                pdt = pstile([128, 32], BF16, tag="sm")
                pdt_v = pdt[:].rearrange("p (i two) -> p i two", two=2)
                for side in range(2):
                    for jj in range(8):
                        r = side * 8 + jj
                        nc.tensor.transpose(
                            pdt_v[:, r : r + 1, 0],
                            uns[side][64:65, jj * 128 : (jj + 1) * 128],
                            ones65[64:65, 0:1],
                        )
                inv16 = invp.tile([128, 16], F32, tag="inv", name="inv")
                nc.vector.reciprocal(inv16[:], pdt_v[:, :, 0])
                inv16b = invp.tile([128, 16], BF16, tag="invb16", name="invb16")
                nc.vector.tensor_copy(inv16b[:], inv16[:])
                ptv = pstile([16, 128], BF16, tag="sm")
                nc.tensor.transpose(ptv[:], inv16b[:], id_sb[:])
                pts = invp.tile([16, 128], BF16, tag="pts", name="pts")
                nc.vector.tensor_copy(pts[:], ptv[:])
                for side in range(2):
                    for n2 in range(2):
                        pinvb = pstile([64, 512], F32, tag="sm")
                        for jj in range(4):
                            r = side * 8 + n2 * 4 + jj
                            nc.tensor.matmul(
                                pinvb[:, jj * 128 : (jj + 1) * 128],
                                selbc_sb[:, r * 64 : (r + 1) * 64],
                                pts[:],
                                start=True,
                                stop=True,
                            )
                        nc.vector.tensor_mul(
                            xap[j // 2][
                                64 * side : 64 * side + 64,
                                j % 2,
                                n2 * 512 : (n2 + 1) * 512,
                            ],
                            uns[side][0:64, n2 * 512 : (n2 + 1) * 512],
                            pinvb[:],
                        )

            for j in range(4):
                # exp-weight tiles for this pair: per (side, t):
                # fp8 [128, 2(s), 1024]; or f32 z-tiles for bf16 fallback
                if EXPBF:
                    ex = [
                        [
                            exq.tile(
                                [128, 2, 2, 512], F32,
                                tag=f"ex{side}{t}", name=f"ex{side}{t}", bufs=1,
                            )
                            for t in range(4)
                        ]
                        for side in range(2)
                    ]
                else:
                    ex = [
                        [
                            exq.tile(
                                [128, 2, NSP], FP8,
                                tag=f"ex{side}{t}", name=f"ex{side}{t}",
                            )
                            for t in range(4)
                        ]
                        for side in range(2)
                    ]
                uns = [
                    unp.tile([65, NSP], BF16, tag=f"un{side}", name=f"un{side}")
                    for side in range(2)
                ]

                def av_pass(n2):
                    for side in range(2):
                        hh = 2 * j + side
                        pav = pstile([66, 512], F32, tag="sm")
                        if EXPBF:
                            for t in range(4):
                                for s2 in range(2):
                                    rhs = (
                                        ex[side][t][:]
                                        .bitcast(BF16)
                                        .rearrange(
                                            "p a b (n two) -> p a b n two", two=2
                                        )[:, s2, n2, :, 0]
                                    )
                                    nc.tensor.matmul(
                                        pav[:],
                                        vts[t][:, s2, 80 * hh : 80 * hh + 66],
                                        rhs,
                                        start=(t == 0 and s2 == 0),
                                        stop=(t == 3 and s2 == 1),
                                        skip_group_check=True,
                                    )
                        else:
                            for t in range(4):
                                nc.tensor.matmul(
                                    pav[:],
                                    vts[t][:, :, 80 * hh : 80 * hh + 66],
                                    ex[side][t][:, :, n2 * 512 : (n2 + 1) * 512],
                                    start=(t == 0),
                                    stop=(t == 3),
                                    perf_mode=DR,
                                    skip_group_check=True,
                                )
                        nc.scalar.activation(
                            uns[side][:, n2 * 512 : (n2 + 1) * 512],
                            pav[0:65, :],
                            AF.Identity,
                            scale=1.0,
                        )

                for t in range(4):
                    for s2 in range(2):
                        mk = 2 * t + s2
                        mks = slice(mk * 128, (mk + 1) * 128)
                        scs = []
                        for side in range(2):
                            ph = slice(64 * side, 64 * side + 64)
                            sc = pstile([128, NSP], F32, tag="ps")
                            for n2 in range(2):
                                s = slice(n2 * 512, (n2 + 1) * 512)
                                nc.tensor.matmul(
                                    sc[:, s],
                                    qk[4 + j][ph, mks],
                                    qk[j][ph, s],
                                    start=True,
                                    stop=True,
                                )
                            scs.append(sc)
                        # exp: side A on ACT (fp8 direct), side B on DVE
                        # (schraudolph bits via f32->int8 round)
                        if EXPBF:
                            nc.scalar.activation(
                                ex[0][t][:, s2, :, :].rearrange("p a b -> p (a b)")
                                .bitcast(BF16)
                                .rearrange("p (n two) -> p n two", two=2)[:, :, 0],
                                scs[0][:],
                                AF.Exp,
                                scale=0.125,
                            )
                            nc.vector.tensor_scalar(
                                ex[1][t][:, s2, :, :].rearrange(
                                    "p a b -> p (a b)"
                                ),
                                scs[1][:],
                                scalar1=EXPA16,
                                scalar2=EXPB16,
                                op0=MULT,
                                op1=ADD,
                            )
                        else:
                            nc.scalar.activation(
                                ex[0][t][:, s2, :], scs[0][:], AF.Exp, scale=0.125
                            )
                            nc.vector.tensor_scalar(
                                ex[1][t][:, s2, :].bitcast(I8),
                                scs[1][:],
                                scalar1=EXPA8,
                                scalar2=EXPB8,
                                op0=MULT,
                                op1=ADD,
                            )
                # AV passes: n2=0 then n2=1 (ex tiles persist in SBUF)
                av_pass(0)
                av_pass(1)
                denom_chain(uns, j)

            if KSTAGE == 3:
                dump_and_finish([xap[t // 2][:, t % 2, :] for t in range(CT)])
                return nc

            load_late_weights()

            # ---- out projection (fp8 DoubleRow; keep f32 for GN2 stats) ----
            attn2 = [
                pers.tile([128, NSP], F32, tag=f"attn2{t}", name=f"attn2{t}")
                for t in range(CT)
            ]
            for m in range(CT):
                ps = pstile([128, NSP], F32)
                for p in range(2):
                    for n2 in range(2):
                        s = slice(n2 * 512, (n2 + 1) * 512)
                        nc.tensor.matmul(
                            ps[:, s],
                            wo_sb[p][:, :, m * 128 : (m + 1) * 128],
                            xap[p][:, :, s],
                            start=(p == 0),
                            stop=(p == 1),
                            perf_mode=DR,
                            skip_group_check=True,
                        )
                nc.scalar.activation(
                    attn2[m][:], ps[:], AF.Identity,
                    bias=outb_sb[:, m : m + 1], scale=1.0 / WS,
                )

            if KSTAGE == 4:
                for t in range(CT):
                    nc.sync.dma_start(out_d[t * 128 : (t + 1) * 128, :], attn2[t][:])
                return nc

            # ---- GN2 -> xg (fp8 pair tiles) ----
            xgp = [
                pers.tile([128, 2, NSP], FP8, tag=f"xgp{p}", name=f"xgp{p}")
                for p in range(2)
            ]
            xg_aps = [xgp[t // 2][:, t % 2, :] for t in range(CT)]
            group_norm(attn2, g2_sb, b2_sb, xg_aps)

            if KSTAGE == 5:
                dump_and_finish(xg_aps, reuse=attn2, scale=1.0)
                return nc

            # ---- MLP1 + SwiGLU (fp8 DoubleRow; h1 x64 descaled in Silu's
            # scale, gate x4 compensated in the MLP2 readout) ----
            actp = [
                pers.tile([128, 2, NSP], FP8, tag=f"actp{p}", name=f"actp{p}")
                for p in range(8)
            ]
            for mp in range(16):
                ps1 = pstile([128, NSP], F32)
                for p in range(2):
                    for n2 in range(2):
                        s = slice(n2 * 512, (n2 + 1) * 512)
                        nc.tensor.matmul(
                            ps1[:, s],
                            w1_sb[p][:, :, mp * 128 : (mp + 1) * 128],
                            xgp[p][:, :, s],
                            start=(p == 0),
                            stop=(p == 1),
                            perf_mode=DR,
                            skip_group_check=True,
                        )
                ps2 = pstile([128, NSP], F32)
                for p in range(2):
                    for n2 in range(2):
                        s = slice(n2 * 512, (n2 + 1) * 512)
                        nc.tensor.matmul(
                            ps2[:, s],
                            w1_sb[p][:, :, (mp + 16) * 128 : (mp + 17) * 128],
                            xgp[p][:, :, s],
                            start=(p == 0),
                            stop=(p == 1),
                            perf_mode=DR,
                            skip_group_check=True,
                        )
                sg = swp.tile([128, NSP], BF16, tag="sw", name="sw")
                nc.scalar.activation(sg[:], ps1[:], AF.Silu, scale=1.0 / WS)
                nc.vector.tensor_mul(actp[mp // 2][:, mp % 2, :], sg[:], ps2[:])

            if KSTAGE == 6:
                dump_and_finish(
                    [actp[t // 2][:, t % 2, :] for t in range(CT)],
                    reuse=attn2, scale=1.0 / GS,
                )
                return nc

            # reload x into the attn2 slots (attention result consumed by GN2)
            for t in range(CT):
                nc.sync.dma_start(attn2[t][:], x_d[t * 128 : (t + 1) * 128, :])

            # ---- MLP2 + residual -> out (fp8 DoubleRow) ----
            for m in range(CT):
                ps = pstile([128, NSP], F32)
                for p in range(8):
                    for n2 in range(2):
                        s = slice(n2 * 512, (n2 + 1) * 512)
                        nc.tensor.matmul(
                            ps[:, s],
                            w2_sb[p][:, :, m * 128 : (m + 1) * 128],
                            actp[p][:, :, s],
                            start=(p == 0),
                            stop=(p == 7),
                            perf_mode=DR,
                            skip_group_check=True,
                        )
                tmp = swp.tile([128, NSP], F32, tag="mlp2t", name="mlp2t")
                nc.scalar.activation(tmp[:], ps[:], AF.Copy, scale=1.0 / (WS * GS))
                nc.vector.tensor_tensor(attn2[m][:], tmp[:], attn2[m][:], op=ADD)
                nc.sync.dma_start(out_d[m * 128 : (m + 1) * 128, :], attn2[m][:])

    return nc


def _get_nc():
    key = ("nc", KSTAGE, EXPBF)
    if key not in _cache:
        import concourse.bass  # noqa: F401  ensure importable before build
        from concourse import mybir

        res = _build_nc()
        nc = res[0] if isinstance(res, tuple) else res
        _split_multi_waits(nc, mybir, maxw=1)
        _cache[key] = nc
    return _cache[key]


def _pair_pack(wT, pairs):
    """wT: [K, M] -> list of `pairs` arrays [128, 2*M] with
    [q, 2s+m]... = wT[256p+128s+q, m] laid out [128][2][M] contiguously."""
    K, M = wT.shape
    assert K == pairs * 256
    out = []
    for p in range(pairs):
        blk = wT[p * 256 : (p + 1) * 256].reshape(2, 128, M).transpose(1, 0, 2)
        out.append(np.ascontiguousarray(blk.reshape(128, 2 * M)))
    return out


def _prep_weights(inputs):
    bf = ml_dtypes.bfloat16
    f8 = ml_dtypes.float8_e4m3
    f32 = np.float32

    def col4(v):  # (512,) -> (128, 4) with [p, t] = v[128t + p]
        return np.ascontiguousarray(v.reshape(4, 128).T.astype(f32))

    qkv_b = inputs["qkv_b"].astype(f32)
    sel = np.zeros((C, 32), f32)
    sel[np.arange(C), np.arange(C) // 16] = 1.0
    selbc = np.zeros((16, 1024), f32)
    for r in range(16):
        selbc[r, r * 64 : (r + 1) * 64] = 1.0
    selbc = selbc.astype(bf)

    wqkvT = np.ascontiguousarray(inputs["qkv_w"].astype(f32).T)
    wqkT = wqkvT[:, 0 : 2 * C] * WS
    wvT = wqkvT[:, 2 * C :] * WS
    woT = np.ascontiguousarray(inputs["out_w"].astype(f32).T) * WS
    w1T = np.ascontiguousarray(inputs["mlp1_w"].astype(f32).T).copy()
    w1T[:, 0:HID] *= WS
    w1T[:, HID:] *= GS
    w2T = np.ascontiguousarray(inputs["mlp2_w"].astype(f32).T) * WS

    # v bias folded into out-proj bias (softmax rows sum to 1)
    ob = inputs["out_b"].astype(f32) + inputs["out_w"].astype(f32) @ qkv_b[2 * C :]

    shared = {
        "qkb": np.ascontiguousarray(qkv_b[0 : 2 * C].reshape(8, 128).T.astype(f32)),
        "outb": col4(ob),
        "g1": col4(inputs["gn1_gamma"].astype(f32)),
        "b1": col4(inputs["gn1_beta"].astype(f32)),
        "g2": col4(inputs["gn2_gamma"].astype(f32)),
        "b2": col4(inputs["gn2_beta"].astype(f32)),
        "sel": sel.astype(bf),
        "selT": np.ascontiguousarray(sel.T).astype(bf),
        "ident": np.eye(128, dtype=f32).astype(bf),
        "selbc": selbc,
    }
    for p, a in enumerate(_pair_pack(wqkT, 2)):
        shared[f"wqkT{p}"] = a.astype(f8)
    for p, a in enumerate(_pair_pack(wvT, 2)):
        shared[f"wvT{p}"] = a.astype(f8)
    for p, a in enumerate(_pair_pack(woT, 2)):
        shared[f"woT{p}"] = a.astype(f8)
    for p, a in enumerate(_pair_pack(w1T, 2)):
        shared[f"w1T{p}"] = a.astype(f8)
    for p, a in enumerate(_pair_pack(w2T, 8)):
        shared[f"w2T{p}"] = a.astype(f8)
    return shared


def kernel(**inputs):
    from concourse.bass_utils import run_bass_kernel_spmd

    nc = _get_nc()
    shared = _prep_weights(inputs)
    x = np.asarray(inputs["x"], dtype=np.float32).reshape(8, C, NSP)
    in_maps = [dict(shared, x=np.ascontiguousarray(x[i])) for i in range(8)]
    res = run_bass_kernel_spmd(nc, in_maps, core_ids=list(range(8))).results
    out = np.stack([res[i]["out"] for i in range(8)], axis=0)
    return out.reshape(8, C, 32, 32).astype(np.float32)
